# revision 16
# baseline (speedup 1.0000x reference)
"""DiffuseRouter kernel for 8 TRN2 NeuronCores.

Reference computation (enable_time=False, soft_time_routing=True):
    out[b, l, d] = (1/3) * sum_g sum_e expert_emb_g[e, b, l, d]
i.e. a uniform-weighted sum of 28 expert planes per batch element.

Sharding: pure data-parallel over batch B=8 -> one batch element per core.
Each core reads its 28 [256, 1280] f32 planes (36.7 MB), reduces them
on-chip, scales by 1/3, and writes its [256, 1280] output.  No collectives.

v12 = window-major streaming with all-uniform loads.  Hard-won trace laws:
  * Only [128, N] dma_starts with a contiguous DRAM block run at line rate
    (~25.9 GB/s per SDMA engine, 413 GB/s wall).  Partial-partition loads
    ([120, N], [32, N]) run at HALF rate (12.8-15 GB/s per engine) -- probe
    measured; that sank v10/v11's engine-15-deload scheme.  Strided DRAM
    sources are worse still (12 GB/s, v9).
  * An all-PE reduction (tensor engine ~44% active) trips the activity
    throttle (util cap 0.5) and drags the DMA fabric ~13%.  The v8 split
    (PE 3 PSUM banks + DVE for 1024 cols, ~33%+28% active) stays clean.
  * SDMA engine 15 has an intermittent ~21.4 GB/s mode (hit in both v8
    runs, absent in v10/v11).  With uniform loads it cannot be deloaded;
    this build accepts the lottery.
Layout: column-window-major streams so output stores overlap the load
stream instead of trailing it.  PE windows w=0,1,2 (cols w*512..w*512+512
of every plane) are host-packed as contiguous chunk strings and
accumulated into PSUM bank w by 28 identity matmuls (fp32r); each bank
stops, scales x1/3 through ACT, and stores while later streams are still
loading.  The DVE zone (cols 1536:2560 of every plane) streams as its own
chunk string, accumulated by DVE scalar_tensor_tensor into SBUF, scaled
and stored at the end (~3 us tail).  Load tiles interleave PE/DVE streams
2:1 so both consumers stay fed at v8-like duty cycles.
"""

import os

# Tile's subtile dependency tracking allocates a semaphore per chunk-read
# edge (~250 for this kernel); the framework epilogue then clears each one
# individually (~5-7 us of teardown).  Consumption here is gated on whole
# tiles (one DMA sem each), so coarse deps lose nothing.  Must be set
# before concourse.env's @cache reads it.
os.environ.setdefault("BY_DEFAULT_DISABLE_SUBTILE_DEPS", "1")

import numpy as np

import concourse.bacc as bacc
import concourse.tile as tile
from concourse import mybir
from concourse.alu_op_type import AluOpType
from concourse.bass_utils import run_bass_kernel_spmd

N_CORES = 8
E_TOTAL = 28  # 4 + 8 + 16 experts across the 3 granularity levels
L, D = 256, 1280
P = 128  # SBUF partitions
FD = (L // P) * D  # 2560 free-dim elements per partition per plane
BW = 512  # one 2 KB PSUM bank of f32
NB_PE = 3  # PE windows (cols 0..1536)
DVE_LO = NB_PE * BW  # 1536
DVE_W = FD - DVE_LO  # 1024 cols accumulated on DVE
SCALE = 1.0 / 3.0

# Tile schedule: (kind, stream, chunk_lo, chunk_hi).  kind 'P' tiles hold
# chunks of 512 cols for PE stream `stream`; kind 'V' tiles hold chunks of
# 1024 cols for the DVE zone.  PE tiles carry 10/10/8 chunks (20 KB/16 KB
# descriptors), DVE tiles 5/3 chunks.  Interleaved ~2:1 to keep both
# engines fed; all loads are full-128-partition contiguous blocks.
SCHED = [
    ("P", 0, 0, 10), ("V", 0, 0, 5), ("P", 0, 10, 20), ("V", 0, 5, 10),
    ("P", 0, 20, 28), ("V", 0, 10, 15), ("P", 1, 0, 10), ("V", 0, 15, 20),
    ("P", 1, 10, 20), ("V", 0, 20, 25), ("P", 1, 20, 28), ("V", 0, 25, 28),
    # Last stream tapers to tiny tiles: consumption is gated per-tile on
    # the tile's LAST descriptor (slow engine 15 sets that pace), so the
    # final tiles carry 2/1/1 chunks to shrink the post-landing serial
    # matmul tail from ~6 us to <1 us.
    ("P", 2, 0, 10), ("P", 2, 10, 20), ("P", 2, 20, 24),
    ("P", 2, 24, 26), ("P", 2, 26, 27), ("P", 2, 27, 28),
]

_NC_CACHE = None


def _build_nc():
    """Build the SPMD Bass program (identical on all 8 cores)."""
    nc = bacc.Bacc(
        "TRN2", target_bir_lowering=False, debug=False, enable_partition_id=False
    )
    f32 = mybir.dt.float32
    f32r = mybir.dt.float32r

    xs, xs_r = [], []
    for i, (kind, s, lo, hi) in enumerate(SCHED):
        cw = BW if kind == "P" else DVE_W
        t = nc.dram_tensor(f"x{i}", [P, (hi - lo) * cw], f32,
                           kind="ExternalInput")
        xs.append(t)
        xs_r.append(t.ap().bitcast(f32r))
    ident_d = nc.dram_tensor("ident", [P, P], f32, kind="ExternalInput")
    out_pe = [
        nc.dram_tensor(f"out{w}", [P, BW], f32, kind="ExternalOutput")
        for w in range(NB_PE)
    ]
    out_dve = nc.dram_tensor("outv", [P, DVE_W], f32, kind="ExternalOutput")

    with tile.TileContext(nc) as tc:
        with (
            tc.tile_pool(name="in", bufs=4) as pin,
            tc.tile_pool(name="const", bufs=1) as pconst,
            tc.tile_pool(name="acc", bufs=1) as pacc,
            tc.tile_pool(name="ps", bufs=1, space="PSUM") as pps,
        ):
            ident = pconst.tile([P, P], f32r, name="ident", tag="ident")
            # Identity rides the ACT ring; the sync ring carries only loads.
            nc.scalar.dma_start(out=ident[:], in_=ident_d.ap().bitcast(f32r))

            psums = [
                pps.tile([P, BW], f32, name=f"ps{w}", tag=f"ps{w}")
                for w in range(NB_PE)
            ]
            souts = pacc.tile([P, NB_PE * BW], f32, name="souts", tag="souts")
            acc = pacc.tile([P, DVE_W], f32, name="acc", tag="acc")
            vout = pacc.tile([P, DVE_W], f32, name="vout", tag="vout")

            mult = AluOpType.mult
            add = AluOpType.add
            n_v = 0

            for i, (kind, s, lo, hi) in enumerate(SCHED):
                cw = BW if kind == "P" else DVE_W
                t = pin.tile([P, (hi - lo) * cw], f32r)
                nc.sync.dma_start(out=t[:], in_=xs_r[i])
                if kind == "P":
                    for c in range(lo, hi):
                        nc.tensor.matmul(
                            psums[s][:], ident[:],
                            t[:, (c - lo) * BW : (c - lo + 1) * BW],
                            start=(c == 0), stop=(c == E_TOTAL - 1),
                        )
                    if hi == E_TOTAL:
                        # Bank s complete: scale + store while later
                        # streams are still loading.
                        ws = slice(s * BW, (s + 1) * BW)
                        nc.scalar.mul(souts[:, ws], psums[s][:], SCALE)
                        nc.scalar.dma_start(
                            out=out_pe[s].ap(), in_=souts[:, ws]
                        )
                else:
                    for c in range(lo, hi):
                        src = t[
                            :, (c - lo) * DVE_W : (c - lo + 1) * DVE_W
                        ].bitcast(f32)
                        if c == 0:
                            nc.vector.tensor_scalar_mul(acc[:], src, 1.0)
                        else:
                            nc.vector.scalar_tensor_tensor(
                                acc[:], src, 1.0, acc[:], mult, add
                            )
                    n_v += hi - lo
                    if hi == E_TOTAL:
                        # DVE zone complete: scale on ACT and store.
                        nc.scalar.mul(vout[:], acc[:], SCALE)
                        nc.scalar.dma_start(out=out_dve.ap(), in_=vout[:])
    nc.compile()
    return nc


def _get_nc():
    global _NC_CACHE
    if _NC_CACHE is None:
        _NC_CACHE = _build_nc()
    return _NC_CACHE


def _pack_core(v):
    """v: [28, 128, 2560] planes for one batch element -> input map."""
    im = {"ident": np.eye(P, dtype=np.float32)}
    for i, (kind, s, lo, hi) in enumerate(SCHED):
        if kind == "P":
            blk = v[lo:hi, :, s * BW : (s + 1) * BW]  # [n, 128, 512]
        else:
            blk = v[lo:hi, :, DVE_LO:FD]  # [n, 128, 1024]
        im[f"x{i}"] = np.ascontiguousarray(
            blk.transpose(1, 0, 2).reshape(P, -1)
        )
    return im


def _run(inputs, trace=False, trace_kwargs=None):
    e0 = np.asarray(inputs["expert_emb_0"], dtype=np.float32)
    e1 = np.asarray(inputs["expert_emb_1"], dtype=np.float32)
    e2 = np.asarray(inputs["expert_emb_2"], dtype=np.float32)
    B = e0.shape[1]
    assert B == N_CORES, f"expected B == {N_CORES}, got {B}"

    in_maps = []
    for b in range(B):
        xb_full = np.concatenate([e0[:, b], e1[:, b], e2[:, b]], axis=0)
        v = xb_full.reshape(E_TOTAL, P, FD)
        in_maps.append(_pack_core(v))

    kw = {}
    if trace:
        kw["trace"] = True
        if trace_kwargs:
            kw.update(trace_kwargs)
    try:
        res = run_bass_kernel_spmd(_get_nc(), in_maps, list(range(N_CORES)), **kw)
    except Exception:
        # One retry: transient device errors usually clear on re-dispatch.
        res = run_bass_kernel_spmd(_get_nc(), in_maps, list(range(N_CORES)), **kw)
    outs = []
    for b in range(B):
        full = np.concatenate(
            [res.results[b][f"out{w}"] for w in range(NB_PE)]
            + [res.results[b]["outv"]],
            axis=1,
        )
        outs.append(full.reshape(L, D))
    out = np.stack(outs, axis=0)
    return out.astype(np.float32, copy=False), res


def kernel(**inputs) -> np.ndarray:
    out, _ = _run(inputs, trace=False)
    return out


# revision 18
# speedup vs baseline: 1.0070x; 1.0070x over previous
"""DiffuseRouter kernel for 8 TRN2 NeuronCores.

Reference computation (enable_time=False, soft_time_routing=True):
    out[b, l, d] = (1/3) * sum_g sum_e expert_emb_g[e, b, l, d]
i.e. a uniform-weighted sum of 28 expert planes per batch element.

Sharding: pure data-parallel over batch B=8 -> one batch element per core.
Each core reads its 28 [256, 1280] f32 planes (36.7 MB), reduces them
on-chip, scales by 1/3, and writes its [256, 1280] output.  No collectives.

v12 = window-major streaming with all-uniform loads.  Hard-won trace laws:
  * Only [128, N] dma_starts with a contiguous DRAM block run at line rate
    (~25.9 GB/s per SDMA engine, 413 GB/s wall).  Partial-partition loads
    ([120, N], [32, N]) run at HALF rate (12.8-15 GB/s per engine) -- probe
    measured; that sank v10/v11's engine-15-deload scheme.  Strided DRAM
    sources are worse still (12 GB/s, v9).
  * An all-PE reduction (tensor engine ~44% active) trips the activity
    throttle (util cap 0.5) and drags the DMA fabric ~13%.  The v8 split
    (PE 3 PSUM banks + DVE for 1024 cols, ~33%+28% active) stays clean.
  * SDMA engine 15 has an intermittent ~21.4 GB/s mode (hit in both v8
    runs, absent in v10/v11).  With uniform loads it cannot be deloaded;
    this build accepts the lottery.
Layout: column-window-major streams so output stores overlap the load
stream instead of trailing it.  PE windows w=0,1,2 (cols w*512..w*512+512
of every plane) are host-packed as contiguous chunk strings and
accumulated into PSUM bank w by 28 identity matmuls (fp32r); each bank
stops, scales x1/3 through ACT, and stores while later streams are still
loading.  The DVE zone (cols 1536:2560 of every plane) streams as its own
chunk string, accumulated by DVE scalar_tensor_tensor into SBUF, scaled
and stored at the end (~3 us tail).  Load tiles interleave PE/DVE streams
2:1 so both consumers stay fed at v8-like duty cycles.
"""

import numpy as np

import concourse.bacc as bacc
import concourse.tile as tile
from concourse import mybir
from concourse.alu_op_type import AluOpType
from concourse.bass_utils import run_bass_kernel_spmd

N_CORES = 8
E_TOTAL = 28  # 4 + 8 + 16 experts across the 3 granularity levels
L, D = 256, 1280
P = 128  # SBUF partitions
FD = (L // P) * D  # 2560 free-dim elements per partition per plane
BW = 512  # one 2 KB PSUM bank of f32
NB_PE = 3  # PE windows (cols 0..1536)
DVE_LO = NB_PE * BW  # 1536
DVE_W = FD - DVE_LO  # 1024 cols accumulated on DVE
SCALE = 1.0 / 3.0

# Tile schedule: (kind, stream, chunk_lo, chunk_hi).  kind 'P' tiles hold
# chunks of 512 cols for PE stream `stream`; kind 'V' tiles hold chunks of
# 1024 cols for the DVE zone.  PE tiles carry 14 chunks (28,672 B
# descriptors), DVE tiles 10/8 chunks (40,960/32,768 B) -- probe-measured
# 26.7 GB/s per engine at 40 KB vs 25.9 at 20 KB.  Interleaved so both
# consumers stay fed; all loads are full-128-partition contiguous blocks.
SCHED = [
    ("P", 0, 0, 14), ("V", 0, 0, 10), ("P", 0, 14, 28), ("P", 1, 0, 14),
    ("V", 0, 10, 20), ("P", 1, 14, 28), ("P", 2, 0, 14), ("V", 0, 20, 28),
    # Last stream tapers to tiny tiles: consumption is gated per-tile on
    # the tile's LAST descriptor, so the final tiles carry 2/1/1 chunks to
    # shrink the post-landing serial matmul tail to <1 us.
    ("P", 2, 14, 24), ("P", 2, 24, 26), ("P", 2, 26, 27), ("P", 2, 27, 28),
]

_NC_CACHE = None


def _build_nc():
    """Build the SPMD Bass program (identical on all 8 cores)."""
    nc = bacc.Bacc(
        "TRN2", target_bir_lowering=False, debug=False, enable_partition_id=False
    )
    f32 = mybir.dt.float32
    f32r = mybir.dt.float32r

    xs, xs_r = [], []
    for i, (kind, s, lo, hi) in enumerate(SCHED):
        cw = BW if kind == "P" else DVE_W
        t = nc.dram_tensor(f"x{i}", [P, (hi - lo) * cw], f32,
                           kind="ExternalInput")
        xs.append(t)
        xs_r.append(t.ap().bitcast(f32r))
    ident_d = nc.dram_tensor("ident", [P, P], f32, kind="ExternalInput")
    out_pe = [
        nc.dram_tensor(f"out{w}", [P, BW], f32, kind="ExternalOutput")
        for w in range(NB_PE)
    ]
    out_dve = nc.dram_tensor("outv", [P, DVE_W], f32, kind="ExternalOutput")

    with tile.TileContext(nc) as tc:
        with (
            tc.tile_pool(name="in", bufs=4) as pin,
            tc.tile_pool(name="const", bufs=1) as pconst,
            tc.tile_pool(name="acc", bufs=1) as pacc,
            tc.tile_pool(name="ps", bufs=1, space="PSUM") as pps,
        ):
            ident = pconst.tile([P, P], f32r, name="ident", tag="ident")
            # Identity rides the ACT ring; the sync ring carries only loads.
            nc.scalar.dma_start(out=ident[:], in_=ident_d.ap().bitcast(f32r))

            psums = [
                pps.tile([P, BW], f32, name=f"ps{w}", tag=f"ps{w}")
                for w in range(NB_PE)
            ]
            souts = pacc.tile([P, NB_PE * BW], f32, name="souts", tag="souts")
            acc = pacc.tile([P, DVE_W], f32, name="acc", tag="acc")
            vout = pacc.tile([P, DVE_W], f32, name="vout", tag="vout")

            mult = AluOpType.mult
            add = AluOpType.add
            n_v = 0

            for i, (kind, s, lo, hi) in enumerate(SCHED):
                cw = BW if kind == "P" else DVE_W
                t = pin.tile([P, (hi - lo) * cw], f32r)
                nc.sync.dma_start(out=t[:], in_=xs_r[i])
                if kind == "P":
                    for c in range(lo, hi):
                        nc.tensor.matmul(
                            psums[s][:], ident[:],
                            t[:, (c - lo) * BW : (c - lo + 1) * BW],
                            start=(c == 0), stop=(c == E_TOTAL - 1),
                        )
                    if hi == E_TOTAL:
                        # Bank s complete: scale + store while later
                        # streams are still loading.
                        ws = slice(s * BW, (s + 1) * BW)
                        nc.scalar.mul(souts[:, ws], psums[s][:], SCALE)
                        nc.scalar.dma_start(
                            out=out_pe[s].ap(), in_=souts[:, ws]
                        )
                else:
                    for c in range(lo, hi):
                        src = t[
                            :, (c - lo) * DVE_W : (c - lo + 1) * DVE_W
                        ].bitcast(f32)
                        if c == 0:
                            nc.vector.tensor_scalar_mul(acc[:], src, 1.0)
                        else:
                            nc.vector.scalar_tensor_tensor(
                                acc[:], src, 1.0, acc[:], mult, add
                            )
                    n_v += hi - lo
                    if hi == E_TOTAL:
                        # DVE zone complete: scale on ACT and store.
                        nc.scalar.mul(vout[:], acc[:], SCALE)
                        nc.scalar.dma_start(out=out_dve.ap(), in_=vout[:])
    nc.compile()
    return nc


def _get_nc():
    global _NC_CACHE
    if _NC_CACHE is None:
        _NC_CACHE = _build_nc()
    return _NC_CACHE


def _pack_core(v):
    """v: [28, 128, 2560] planes for one batch element -> input map."""
    im = {"ident": np.eye(P, dtype=np.float32)}
    for i, (kind, s, lo, hi) in enumerate(SCHED):
        if kind == "P":
            blk = v[lo:hi, :, s * BW : (s + 1) * BW]  # [n, 128, 512]
        else:
            blk = v[lo:hi, :, DVE_LO:FD]  # [n, 128, 1024]
        im[f"x{i}"] = np.ascontiguousarray(
            blk.transpose(1, 0, 2).reshape(P, -1)
        )
    return im


def _run(inputs, trace=False, trace_kwargs=None):
    e0 = np.asarray(inputs["expert_emb_0"], dtype=np.float32)
    e1 = np.asarray(inputs["expert_emb_1"], dtype=np.float32)
    e2 = np.asarray(inputs["expert_emb_2"], dtype=np.float32)
    B = e0.shape[1]
    assert B == N_CORES, f"expected B == {N_CORES}, got {B}"

    in_maps = []
    for b in range(B):
        xb_full = np.concatenate([e0[:, b], e1[:, b], e2[:, b]], axis=0)
        v = xb_full.reshape(E_TOTAL, P, FD)
        in_maps.append(_pack_core(v))

    kw = {}
    if trace:
        kw["trace"] = True
        if trace_kwargs:
            kw.update(trace_kwargs)
    try:
        res = run_bass_kernel_spmd(_get_nc(), in_maps, list(range(N_CORES)), **kw)
    except Exception:
        # One retry: transient device errors usually clear on re-dispatch.
        res = run_bass_kernel_spmd(_get_nc(), in_maps, list(range(N_CORES)), **kw)
    outs = []
    for b in range(B):
        full = np.concatenate(
            [res.results[b][f"out{w}"] for w in range(NB_PE)]
            + [res.results[b]["outv"]],
            axis=1,
        )
        outs.append(full.reshape(L, D))
    out = np.stack(outs, axis=0)
    return out.astype(np.float32, copy=False), res


def kernel(**inputs) -> np.ndarray:
    out, _ = _run(inputs, trace=False)
    return out


# revision 20
# speedup vs baseline: 1.1717x; 1.1635x over previous
"""DiffuseRouter kernel for 8 TRN2 NeuronCores.

Reference computation (enable_time=False, soft_time_routing=True):
    out[b, l, d] = (1/3) * sum_g sum_e expert_emb_g[e, b, l, d]
i.e. a uniform-weighted sum of 28 expert planes per batch element.

Sharding: pure data-parallel over batch B=8 -> one batch element per core.
Each core reads its 28 [256, 1280] f32 planes (36.7 MB), reduces them
on-chip, scales by 1/3, and writes its [256, 1280] output.  No collectives.

v12 = window-major streaming with all-uniform loads.  Hard-won trace laws:
  * Only [128, N] dma_starts with a contiguous DRAM block run at line rate
    (~25.9 GB/s per SDMA engine, 413 GB/s wall).  Partial-partition loads
    ([120, N], [32, N]) run at HALF rate (12.8-15 GB/s per engine) -- probe
    measured; that sank v10/v11's engine-15-deload scheme.  Strided DRAM
    sources are worse still (12 GB/s, v9).
  * An all-PE reduction (tensor engine ~44% active) trips the activity
    throttle (util cap 0.5) and drags the DMA fabric ~13%.  The v8 split
    (PE 3 PSUM banks + DVE for 1024 cols, ~33%+28% active) stays clean.
  * SDMA engine 15 has an intermittent ~21.4 GB/s mode (hit in both v8
    runs, absent in v10/v11).  With uniform loads it cannot be deloaded;
    this build accepts the lottery.
Layout: column-window-major streams so output stores overlap the load
stream instead of trailing it.  PE windows w=0,1,2 (cols w*512..w*512+512
of every plane) are host-packed as contiguous chunk strings and
accumulated into PSUM bank w by 28 identity matmuls (fp32r); each bank
stops, scales x1/3 through ACT, and stores while later streams are still
loading.  The DVE zone (cols 1536:2560 of every plane) streams as its own
chunk string, accumulated by DVE scalar_tensor_tensor into SBUF, scaled
and stored at the end (~3 us tail).  Load tiles interleave PE/DVE streams
2:1 so both consumers stay fed at v8-like duty cycles.
"""

import numpy as np

import concourse.bacc as bacc
import concourse.tile as tile
from concourse import mybir
from concourse.alu_op_type import AluOpType
from concourse.bass_utils import run_bass_kernel_spmd

N_CORES = 8
E_TOTAL = 28  # 4 + 8 + 16 experts across the 3 granularity levels
L, D = 256, 1280
P = 128  # SBUF partitions
FD = (L // P) * D  # 2560 free-dim elements per partition per plane
BW = 512  # one 2 KB PSUM bank of f32
NB_PE = 3  # PE windows (cols 0..1536)
DVE_LO = NB_PE * BW  # 1536
DVE_W = FD - DVE_LO  # 1024 cols accumulated on DVE
SCALE = 1.0 / 3.0

# Tile schedule: (kind, stream, chunk_lo, chunk_hi).  kind 'P' tiles hold
# chunks of 512 cols for PE stream `stream`; kind 'V' tiles hold chunks of
# 1024 cols for the DVE zone.  PE tiles carry 14 chunks (28,672 B
# descriptors), DVE tiles 10/8 chunks (40,960/32,768 B) -- probe-measured
# 26.7 GB/s per engine at 40 KB vs 25.9 at 20 KB.  Interleaved so both
# consumers stay fed; all loads are full-128-partition contiguous blocks.
SCHED = [
    ("P", 0, 0, 10), ("V", 0, 0, 5), ("P", 0, 10, 20), ("V", 0, 5, 10),
    ("P", 0, 20, 28), ("V", 0, 10, 15), ("P", 1, 0, 10), ("V", 0, 15, 20),
    ("P", 1, 10, 20), ("V", 0, 20, 25), ("P", 1, 20, 28), ("V", 0, 25, 28),
    # Last stream tapers to tiny tiles: consumption is gated per-tile on
    # the tile's LAST descriptor, so the final tiles carry 2/1/1 chunks to
    # shrink the post-landing serial matmul tail to <1 us.
    ("P", 2, 0, 10), ("P", 2, 10, 20), ("P", 2, 20, 24),
    ("P", 2, 24, 26), ("P", 2, 26, 27), ("P", 2, 27, 28),
]

_NC_CACHE = None


def _build_nc():
    """Build the SPMD Bass program (identical on all 8 cores)."""
    nc = bacc.Bacc(
        "TRN2", target_bir_lowering=False, debug=False, enable_partition_id=False
    )
    f32 = mybir.dt.float32
    f32r = mybir.dt.float32r

    xs, xs_r = [], []
    for i, (kind, s, lo, hi) in enumerate(SCHED):
        cw = BW if kind == "P" else DVE_W
        t = nc.dram_tensor(f"x{i}", [P, (hi - lo) * cw], f32,
                           kind="ExternalInput")
        xs.append(t)
        xs_r.append(t.ap().bitcast(f32r))
    ident_d = nc.dram_tensor("ident", [P, P], f32, kind="ExternalInput")
    out_pe = [
        nc.dram_tensor(f"out{w}", [P, BW], f32, kind="ExternalOutput")
        for w in range(NB_PE)
    ]
    out_dve = nc.dram_tensor("outv", [P, DVE_W], f32, kind="ExternalOutput")

    with tile.TileContext(nc) as tc:
        with (
            tc.tile_pool(name="in", bufs=8) as pin,
            tc.tile_pool(name="const", bufs=1) as pconst,
            tc.tile_pool(name="acc", bufs=1) as pacc,
            tc.tile_pool(name="ps", bufs=1, space="PSUM") as pps,
        ):
            ident = pconst.tile([P, P], f32r, name="ident", tag="ident")
            # Identity rides the ACT ring; the sync ring carries only loads.
            nc.scalar.dma_start(out=ident[:], in_=ident_d.ap().bitcast(f32r))

            psums = [
                pps.tile([P, BW], f32, name=f"ps{w}", tag=f"ps{w}")
                for w in range(NB_PE)
            ]
            souts = pacc.tile([P, NB_PE * BW], f32, name="souts", tag="souts")
            acc = pacc.tile([P, DVE_W], f32, name="acc", tag="acc")
            vout = pacc.tile([P, DVE_W], f32, name="vout", tag="vout")

            mult = AluOpType.mult
            add = AluOpType.add
            n_v = 0

            for i, (kind, s, lo, hi) in enumerate(SCHED):
                cw = BW if kind == "P" else DVE_W
                t = pin.tile([P, (hi - lo) * cw], f32r)
                nc.sync.dma_start(out=t[:], in_=xs_r[i])
                if kind == "P":
                    for c in range(lo, hi):
                        nc.tensor.matmul(
                            psums[s][:], ident[:],
                            t[:, (c - lo) * BW : (c - lo + 1) * BW],
                            start=(c == 0), stop=(c == E_TOTAL - 1),
                        )
                    if hi == E_TOTAL:
                        # Bank s complete: scale + store while later
                        # streams are still loading.
                        ws = slice(s * BW, (s + 1) * BW)
                        nc.scalar.mul(souts[:, ws], psums[s][:], SCALE)
                        nc.scalar.dma_start(
                            out=out_pe[s].ap(), in_=souts[:, ws]
                        )
                else:
                    for c in range(lo, hi):
                        src = t[
                            :, (c - lo) * DVE_W : (c - lo + 1) * DVE_W
                        ].bitcast(f32)
                        if c == 0:
                            nc.vector.tensor_scalar_mul(acc[:], src, 1.0)
                        else:
                            nc.vector.scalar_tensor_tensor(
                                acc[:], src, 1.0, acc[:], mult, add
                            )
                    n_v += hi - lo
                    if hi == E_TOTAL:
                        # DVE zone complete: scale on ACT and store.
                        nc.scalar.mul(vout[:], acc[:], SCALE)
                        nc.scalar.dma_start(out=out_dve.ap(), in_=vout[:])
    nc.compile()
    return nc


def _get_nc():
    global _NC_CACHE
    if _NC_CACHE is None:
        _NC_CACHE = _build_nc()
    return _NC_CACHE


def _pack_core(v):
    """v: [28, 128, 2560] planes for one batch element -> input map."""
    im = {"ident": np.eye(P, dtype=np.float32)}
    for i, (kind, s, lo, hi) in enumerate(SCHED):
        if kind == "P":
            blk = v[lo:hi, :, s * BW : (s + 1) * BW]  # [n, 128, 512]
        else:
            blk = v[lo:hi, :, DVE_LO:FD]  # [n, 128, 1024]
        im[f"x{i}"] = np.ascontiguousarray(
            blk.transpose(1, 0, 2).reshape(P, -1)
        )
    return im


def _run(inputs, trace=False, trace_kwargs=None):
    e0 = np.asarray(inputs["expert_emb_0"], dtype=np.float32)
    e1 = np.asarray(inputs["expert_emb_1"], dtype=np.float32)
    e2 = np.asarray(inputs["expert_emb_2"], dtype=np.float32)
    B = e0.shape[1]
    assert B == N_CORES, f"expected B == {N_CORES}, got {B}"

    in_maps = []
    for b in range(B):
        xb_full = np.concatenate([e0[:, b], e1[:, b], e2[:, b]], axis=0)
        v = xb_full.reshape(E_TOTAL, P, FD)
        in_maps.append(_pack_core(v))

    kw = {}
    if trace:
        kw["trace"] = True
        if trace_kwargs:
            kw.update(trace_kwargs)
    try:
        res = run_bass_kernel_spmd(_get_nc(), in_maps, list(range(N_CORES)), **kw)
    except Exception:
        # One retry: transient device errors usually clear on re-dispatch.
        res = run_bass_kernel_spmd(_get_nc(), in_maps, list(range(N_CORES)), **kw)
    outs = []
    for b in range(B):
        full = np.concatenate(
            [res.results[b][f"out{w}"] for w in range(NB_PE)]
            + [res.results[b]["outv"]],
            axis=1,
        )
        outs.append(full.reshape(L, D))
    out = np.stack(outs, axis=0)
    return out.astype(np.float32, copy=False), res


def kernel(**inputs) -> np.ndarray:
    out, _ = _run(inputs, trace=False)
    return out
